# revision 20
# baseline (speedup 1.0000x reference)
"""Trainium2 Bass kernel for nn_Attention_34351148434119 (8 NeuronCores).

Reference computation (faithful quirks included):
  q_proj = hid @ Wq; q, gate = split(q_proj)     # q is DEAD code downstream
  k = hid @ Wk; v = hid @ Wv                     # [B,KV,S,D]
  v = RoPE(v)  (k is NOT roped; q roped but unused)
  scores = (k @ v^T) * sqrt(D) + mask; attn = softmax_t(scores)   # per kv head
  out = (tile_G(attn @ v) * sigmoid(gate)) @ Wo

Sharding: core = b*4 + j  (b = batch, j = rank in 4-core batch group).
Per batch, S=2048 is split into 16 blocks of 128 rows; core j owns blocks
{j, 4+j, 8+j, 12+j} (slot k block = 4k+j) so every core has an identical
causal workload (uniform SPMD graph; per-core specialization only via
staged data).

Schedule (v4): hi/lo bf16 splits are precomputed on the HOST (hidden
staged as one [HS, 2*ROWS] hi|lo array; Wk/Wv as bf16 hi+lo pairs;
Wq-gate and Wo staged bf16) so projections are pure DMA->PE with ONE
coarse DMA per contraction tile (v3's fine-grained per-head slabs choked
the sync sequencer's ~0.6us/DMA issue rate).  v is projected first
(kk-outer), the four per-head fp32 d-major AllGathers fire ~50us in; the
k projection, gate block 0 and the per-head attention overlap the
collectives.  Gathered-v loads ride the gpsimd queue only.  Row-major v
for attn@v is rebuilt on-chip with bf16 PE transposes of a bf16 cast of
the gathered v.  Softmax keeps attn UNNORMALIZED through the transpose
and attn@v; the 1/sum is applied on the [d,s]-layout attn@v output via
PE-transpose of the reciprocal + gpsimd partition_broadcast.
Consecutive matmuls share the stationary operand where possible
([whi@hidH, whi@hidL, wlo@hidH] triple order skips one weight load).

Precision: logits have sigma~105 (SCALING MULTIPLIES by sqrt(D)), so
softmax is effectively near-argmax and bf16 anywhere in the k/v->scores
chain flips argmax rows (rel err ~0.1).  The k/v projections use the
hi/lo bf16 split (3 passes = ~16-bit effective mantissa at full bf16 PE
speed); scores run in native fp32; gate / attn@v / out-proj run bf16.
"""
import sys
import numpy as np

sys.path.insert(0, "/opt/trn_rl_repo")

B, S, HS = 2, 2048, 2048
H, KV, D = 16, 4, 128
G = H // KV
SCALING = float(D) ** 0.5
P = 128
NB = S // P            # 16 row blocks per batch
NCORES = 8
RANKS = 4              # cores per batch group
SLOTS = 4              # owned 128-row blocks per core
ROWS = SLOTS * P       # 512 rows per core
CHUNK = 512            # t-chunk = 4 t-tiles
NCHUNK = S // CHUNK    # 4
KT = HS // P           # 16 contraction tiles
NEG_THRESH = -1e8

_CACHE = {}


def _mask_classes(mask):
    """Classify each (s-slot k, t-chunk c) 512x512 region of the SxS mask.

    0 = skip (everything <= NEG_THRESH: contributes exact 0 after softmax)
    1 = plain (all zeros: no add needed)
    2 = add  (mixed: stage values and add on-chip)
    Slot k rows across all cores = blocks 4k..4k+3 = rows [512k, 512k+512).
    """
    cls = [[0] * NCHUNK for _ in range(SLOTS)]
    for k in range(SLOTS):
        for c in range(NCHUNK):
            reg = mask[512 * k:512 * (k + 1), 512 * c:512 * (c + 1)]
            if (reg <= NEG_THRESH).all():
                cls[k][c] = 0
            elif (reg == 0).all():
                cls[k][c] = 1
            else:
                cls[k][c] = 2
    ok = True
    for k in range(SLOTS):
        comp = [c for c in range(NCHUNK) if cls[k][c] != 0]
        # computed chunks must be a prefix starting at 0
        if comp != list(range(len(comp))) or 0 not in comp:
            ok = False
    if ok:
        # {k : chunk c computed} must be a suffix of slots for each c
        for c in range(NCHUNK):
            ks = [k for k in range(SLOTS) if cls[k][c] != 0]
            if ks != list(range(SLOTS - len(ks), SLOTS)):
                ok = False
    if not ok:
        # fully dense fallback: always correct for any mask
        cls = [[2] * NCHUNK for _ in range(SLOTS)]
    return cls


def _build(classes):
    from contextlib import ExitStack

    from concourse import bacc, mybir, tile
    from concourse.masks import make_identity

    f32 = mybir.dt.float32
    bf16 = mybir.dt.bfloat16
    Alu = mybir.AluOpType
    Act = mybir.ActivationFunctionType

    computed = [[c for c in range(NCHUNK) if classes[k][c] != 0] for k in range(SLOTS)]
    add_idx = {}
    for k in range(SLOTS):
        for c in range(NCHUNK):
            if classes[k][c] == 2:
                add_idx[(k, c)] = len(add_idx)
    n_add = max(len(add_idx), 1)

    nc = bacc.Bacc("TRN2", target_bir_lowering=False, debug=False,
                   num_devices=NCORES)

    # hidHL: per contraction tile kk, columns [hidH(512) | hidL(512)]
    hidHL_d = nc.declare_dram_parameter("hidHL", [HS, 2 * ROWS], bf16,
                                        isOutput=False)
    wvP_d = nc.declare_dram_parameter("wvP", [HS, 2 * KV * D], bf16,
                                      isOutput=False)
    wkvH_d = nc.declare_dram_parameter("wkvH", [HS, 2 * KV * D], bf16,
                                       isOutput=False)
    wkvL_d = nc.declare_dram_parameter("wkvL", [HS, 2 * KV * D], bf16,
                                       isOutput=False)
    wqg_d = nc.declare_dram_parameter("wqg", [HS, HS], bf16, isOutput=False)
    wo_d = nc.declare_dram_parameter("wo", [HS, HS], bf16, isOutput=False)
    cosT_d = nc.declare_dram_parameter("cosT", [D, ROWS], f32, isOutput=False)
    sinT_d = nc.declare_dram_parameter("sinT", [D, ROWS], f32, isOutput=False)
    mask_d = nc.declare_dram_parameter("maskst", [n_add, P, CHUNK], f32,
                                       isOutput=False)
    out_d = nc.declare_dram_parameter("out", [ROWS, HS], bf16, isOutput=True)

    rg = [[0, 1, 2, 3], [4, 5, 6, 7]]

    with tile.TileContext(nc) as tc, ExitStack() as ctx:
        sb = ctx.enter_context(tc.tile_pool(name="sb", bufs=2))
        ps = ctx.enter_context(tc.tile_pool(name="ps", bufs=8, space="PSUM"))
        dram = ctx.enter_context(tc.tile_pool(name="dram", bufs=1, space="DRAM"))

        # ---- constants ----
        id_f32 = sb.tile([P, P], f32, tag="c_idf")
        id_bf = sb.tile([P, P], bf16, tag="c_idb")
        make_identity(nc, id_f32[:])
        make_identity(nc, id_bf[:])
        cosT = sb.tile([D, ROWS], f32, tag="c_cos")
        sinT = sb.tile([D, ROWS], f32, tag="c_sin")
        msk = []
        for i in range(n_add):
            t = sb.tile([P, CHUNK], f32, tag="msk", bufs=n_add, name=f"msk{i}")
            nc.scalar.dma_start(t[:], mask_d[i, :, :])
            msk.append(t)

        # ---- resident hidden (hi|lo), first on the sync queue ----
        hid = []       # [128, 1024] per kk: cols [H(512) | L(512)], resident
        for kk in range(KT):
            hh = sb.tile([P, 2 * ROWS], bf16, tag="hid", bufs=KT,
                         name=f"hid{kk}")
            nc.sync.dma_start(hh[:], hidHL_d[kk * P:(kk + 1) * P, :])
            hid.append(hh)
            if kk == 3:
                nc.sync.dma_start(cosT[:], cosT_d[:, :])
                nc.sync.dma_start(sinT[:], sinT_d[:, :])

        # ---- v projection (g-OUTER, 3-pass hi/lo bf16) + per-head AG ----
        # wvP columns: per head g, 256 cols = [whi(128) | wlo(128)]
        vt_all = []
        for g in range(KV):
            pvg = ps.tile([P, ROWS], f32, tag="ps", name=f"pv{g}")
            for kk in range(KT):
                wv = sb.tile([P, 2 * P], bf16, tag="wvg", bufs=8,
                             name=f"wvg{g}_{kk}")
                eng = nc.sync if kk % 2 == 0 else nc.scalar
                eng.dma_start(wv[:], wvP_d[kk * P:(kk + 1) * P,
                                           g * 2 * P:(g + 1) * 2 * P])
                hH = hid[kk][:, 0:ROWS]
                hL = hid[kk][:, ROWS:2 * ROWS]
                # whi shared by the first two matmuls (saves a weight load)
                nc.tensor.matmul(pvg[:], wv[:, 0:P], hH,
                                 start=(kk == 0), stop=False)
                nc.tensor.matmul(pvg[:], wv[:, 0:P], hL,
                                 start=False, stop=False)
                nc.tensor.matmul(pvg[:], wv[:, P:2 * P], hH,
                                 start=False, stop=(kk == KT - 1))
            vr = sb.tile([P, ROWS], f32, tag="vraw", bufs=2, name=f"vr{g}")
            nc.scalar.copy(vr[:], pvg[:])
            rot = sb.tile([P, ROWS], f32, tag="vrot", bufs=2, name=f"rot{g}")
            nc.vector.tensor_scalar_mul(rot[0:64, :], vr[64:128, :], -1.0)
            nc.vector.tensor_copy(rot[64:128, :], vr[0:64, :])
            nc.vector.tensor_mul(vr[:], vr[:], cosT[:])
            nc.vector.tensor_mul(rot[:], rot[:], sinT[:])
            nc.vector.tensor_add(vr[:], vr[:], rot[:])
            vt_in = dram.tile([P, ROWS], f32, name=f"vtin{g}")
            nc.sync.dma_start(vt_in[:], vr[:])
            vt_out = dram.tile([RANKS, P, ROWS], f32, name=f"vtout{g}")
            nc.gpsimd.collective_compute(
                "AllGather", mybir.AluOpType.bypass, replica_groups=rg,
                ins=[vt_in.opt()], outs=[vt_out.opt()])
            vt_all.append(vt_out)

        # ---- k projection (kk-outer, 3-pass hi/lo bf16) ----
        pk = [ps.tile([P, ROWS], f32, tag="ps", name=f"pk{g}") for g in range(KV)]
        wqb0 = []
        for kk in range(KT):
            wk = sb.tile([P, 2 * KV * D], bf16, tag="wks", bufs=4,
                         name=f"wks{kk}")
            nc.sync.dma_start(wk[:, 0:KV * D],
                              wkvH_d[kk * P:(kk + 1) * P, KV * D:])
            nc.sync.dma_start(wk[:, KV * D:],
                              wkvL_d[kk * P:(kk + 1) * P, KV * D:])
            if kk < 8:
                bs = sb.tile([P, CHUNK], bf16, tag="wqslab", bufs=8,
                             name=f"wqb0_{kk}")
                nc.sync.dma_start(bs[:], wqg_d[kk * P:(kk + 1) * P, 0:CHUNK])
                wqb0.append(bs)
            hH = hid[kk][:, 0:ROWS]
            hL = hid[kk][:, ROWS:2 * ROWS]
            for g in range(KV):
                hi = slice(g * P, (g + 1) * P)
                lo = slice(KV * D + g * P, KV * D + (g + 1) * P)
                nc.tensor.matmul(pk[g][:], wk[:, hi], hH,
                                 start=(kk == 0), stop=False)
                nc.tensor.matmul(pk[g][:], wk[:, hi], hL,
                                 start=False, stop=False)
                nc.tensor.matmul(pk[g][:], wk[:, lo], hH,
                                 start=False, stop=(kk == KT - 1))
        kT = []
        for g in range(KV):
            t = sb.tile([P, ROWS], f32, tag="kT", bufs=KV, name=f"kT{g}")
            nc.scalar.mul(t[:], pk[g][:], SCALING)
            kT.append(t)

        # ---- gate matmul helper (bf16 weights staged from host) ----
        sigT = [None] * 16

        def gate_block(nblk):
            pgs = [ps.tile([P, ROWS], f32, tag="ps", name=f"pg{nblk}_{m}")
                   for m in range(4)]
            for kk in range(KT):
                if nblk == 0 and kk < 8:
                    bs = wqb0[kk]
                else:
                    bs = sb.tile([P, CHUNK], bf16, tag="wqslab", bufs=8,
                                 name=f"wqbx{nblk}_{kk}")
                    nc.sync.dma_start(
                        bs[:], wqg_d[kk * P:(kk + 1) * P,
                                     nblk * CHUNK:(nblk + 1) * CHUNK])
                for m in range(4):
                    nc.tensor.matmul(pgs[m][:], bs[:, m * P:(m + 1) * P],
                                     hid[kk][:, 0:ROWS], start=(kk == 0),
                                     stop=(kk == KT - 1))
            for m in range(4):
                t = sb.tile([P, ROWS], bf16, tag="sigT", bufs=16,
                            name=f"sig{nblk}_{m}")
                nc.scalar.activation(t[:], pgs[m][:], Act.Sigmoid)
                sigT[nblk * 4 + m] = t

        # gate block 0 fills the PE while the first AllGather completes
        gate_block(0)

        # ---- attention per kv head, software-pipelined ----
        # Stages per unit u=(g,k):
        #   S1: score matmuls (PE) + mask add + chunk maxes + min chain (DVE)
        #   S2: exp wave + sum chain (ACT only)
        #   S3: PE transposes + PSUM->SBUF copies into attnT
        #   S4 (per g): attn@v, 1/sum reciprocal+broadcast, normalized avT
        # Emission is pipelined (S1(u) | S2(u-1) | S3(u-2)) so each engine's
        # FIFO sees early-stage ops of later units BEFORE late-stage ops of
        # earlier units -- without this the 16 units run back-to-back
        # serially (~11us latency each).
        avT = [None] * KV
        order = sorted(range(SLOTS), key=lambda k: -len(computed[k]))

        def load_v(g):
            # load gathered v (d-major fp32): vtc[c] = [128 d, 512 t]
            # t-block 4c+r lives in rank r's AG block at column-slot c.
            vtc = []
            vtcb = []
            for c in range(NCHUNK):
                t = sb.tile([P, CHUNK], f32, tag="vtc", bufs=8,
                            name=f"vtc{g}_{c}")
                nc.gpsimd.dma_start(
                    t[:],
                    vt_all[g][:, :, c * P:(c + 1) * P].rearrange(
                        "r d t -> d r t"))
                vtc.append(t)
                tb = sb.tile([P, CHUNK], bf16, tag="vtcb", bufs=8,
                             name=f"vtcb{g}_{c}")
                if (g + c) % 2:
                    nc.scalar.copy(tb[:], t[:])
                else:
                    nc.vector.tensor_copy(tb[:], t[:])
                vtcb.append(tb)
            # row-major bf16 v via PE transpose of the bf16 cast
            vrg = []
            for c in range(NCHUNK):
                tp = ps.tile([P, CHUNK], bf16, tag="ps", name=f"vtp{g}_{c}")
                for r in range(RANKS):
                    nc.tensor.transpose(tp[:, r * P:(r + 1) * P],
                                        vtcb[c][:, r * P:(r + 1) * P], id_bf[:])
                for r in range(RANKS):
                    t = sb.tile([P, P], bf16, tag="vrg", bufs=4 * NCHUNK * 2,
                                name=f"vrg{g}_{4 * c + r}")
                    if r % 2:
                        nc.scalar.copy(t[:], tp[:, r * P:(r + 1) * P])
                    else:
                        nc.vector.tensor_copy(t[:], tp[:, r * P:(r + 1) * P])
                    vrg.append(t)
            return vtc, vtcb, vrg

        def s1_scores(g, k, vtc):
            comp = computed[k]
            pscs = []
            cms = []
            for ci, c in enumerate(comp):
                psc = ps.tile([P, CHUNK], f32, tag="ps",
                              name=f"psc{g}_{k}_{ci}")
                nc.tensor.matmul(psc[:], kT[g][:, k * P:(k + 1) * P],
                                 vtc[c][:], start=True, stop=True)
                if classes[k][c] == 2:
                    nc.vector.tensor_add(psc[:], psc[:],
                                         msk[add_idx[(k, c)]][:])
                cm = sb.tile([P, 1], f32, tag="stat", bufs=96,
                             name=f"cm{g}_{k}_{ci}")
                nc.vector.tensor_reduce(cm[:], psc[:], mybir.AxisListType.X,
                                        Alu.max, negate=True)
                pscs.append(psc)
                cms.append(cm)
            mneg = cms[0]   # -max
            for ci in range(1, len(comp)):
                mnew = sb.tile([P, 1], f32, tag="stat", bufs=96,
                               name=f"mn{g}_{k}_{ci}")
                nc.vector.tensor_tensor(mnew[:], mneg[:], cms[ci][:], Alu.min)
                mneg = mnew
            return pscs, mneg

        def s2_exp(g, k, pscs, mneg):
            comp = computed[k]
            nchk = len(comp)
            attn = sb.tile([P, CHUNK * nchk], bf16, tag="attn", bufs=3,
                           padded_shape=[P, CHUNK * NCHUNK],
                           name=f"attn{g}_{k}")
            tot = None
            for ci in range(nchk):
                csum = sb.tile([P, 1], f32, tag="stat", bufs=96,
                               name=f"cs{g}_{k}_{ci}")
                nc.scalar.activation(attn[:, ci * CHUNK:(ci + 1) * CHUNK],
                                     pscs[ci][:], Act.Exp, bias=mneg[:],
                                     accum_out=csum[:])
                if tot is None:
                    tot = csum
                else:
                    t2 = sb.tile([P, 1], f32, tag="stat", bufs=96,
                                 name=f"tt{g}_{k}_{ci}")
                    nc.scalar.add(t2[:], csum[:], tot[:])
                    tot = t2
            return attn, tot

        def s3_transpose(g, k, attn, attnT):
            comp = computed[k]
            for ci, c in enumerate(comp):
                tp = ps.tile([P, CHUNK], bf16, tag="ps",
                             name=f"atp{g}_{k}_{ci}")
                for i in range(4):
                    nc.tensor.transpose(
                        tp[:, i * P:(i + 1) * P],
                        attn[:, ci * CHUNK + i * P:ci * CHUNK + (i + 1) * P],
                        id_bf[:])
                for i in range(4):
                    bi = 4 * c + i
                    if (ci + i) % 2:
                        nc.scalar.copy(attnT[bi][:, k * P:(k + 1) * P],
                                       tp[:, i * P:(i + 1) * P])
                    else:
                        nc.vector.tensor_copy(attnT[bi][:, k * P:(k + 1) * P],
                                              tp[:, i * P:(i + 1) * P])

        def s4_av(g, attnT, vrg, tots):
            # attn @ v  ->  pav [128 d, 512 s]  (unnormalized)
            pav = ps.tile([P, ROWS], f32, tag="ps", name=f"pav{g}")
            first = True
            for bi in range(NB):
                ks = [k for k in range(SLOTS) if (bi // RANKS) in computed[k]]
                if not ks:
                    continue
                kmin = ks[0]
                nc.tensor.matmul(pav[:, kmin * P:ROWS], vrg[bi][:],
                                 attnT[bi][:, kmin * P:ROWS],
                                 start=first, stop=(bi == NB - 1))
                first = False
            # 1/sum: PE-transpose rinv [128,1] -> [1,128], gpsimd broadcast
            # to [128,128], multiply into the PSUM->SBUF copy of attn@v.
            t = sb.tile([P, ROWS], bf16, tag="avT", bufs=KV, name=f"avT{g}")
            for k in range(SLOTS):
                rinv = sb.tile([P, 1], f32, tag="stat", bufs=96,
                               name=f"rinv{g}_{k}")
                nc.vector.reciprocal(rinv[:], tots[k][:])
                rtp = ps.tile([1, P], f32, tag="ps", name=f"rtp{g}_{k}")
                nc.tensor.transpose(rtp[:], rinv[:], id_f32[:])
                rrow = sb.tile([1, P], f32, tag="rrow", bufs=8,
                               name=f"rrow{g}_{k}")
                nc.vector.tensor_copy(rrow[:], rtp[:])
                rbs = sb.tile([P, P], f32, tag="rbs", bufs=4, name=f"rbs{g}_{k}")
                nc.gpsimd.partition_broadcast(rbs[:], rrow[:])
                nc.vector.tensor_mul(t[:, k * P:(k + 1) * P],
                                     pav[:, k * P:(k + 1) * P], rbs[:])
            avT[g] = t

        units = [(g, k) for g in range(KV) for k in order]
        NU = len(units)
        vload = [None] * KV
        vload[0] = load_v(0)
        attnTs = {}
        s1st = {}
        s2st = {}
        tots = {}
        for i in range(NU + 2):
            if i < NU:
                g, k = units[i]
                if k == order[0]:
                    attnTs[g] = [
                        sb.tile([P, ROWS], bf16, tag="attnT", bufs=NB,
                                name=f"attnT{g}_{bi}") for bi in range(NB)]
                if k == order[-1] and g + 1 < KV:
                    vload[g + 1] = load_v(g + 1)
                s1st[i] = s1_scores(g, k, vload[g][0])
            if 1 <= i <= NU:
                g, k = units[i - 1]
                s2st[i - 1] = s2_exp(g, k, *s1st[i - 1])
            if 2 <= i <= NU + 1:
                g, k = units[i - 2]
                attn, tot = s2st[i - 2]
                s3_transpose(g, k, attn, attnTs[g])
                tots.setdefault(g, {})[k] = tot
                if k == order[-1]:
                    s4_av(g, attnTs[g], vload[g][2], tots[g])

        # ---- remaining gate blocks ----
        for nblk in range(1, 4):
            gate_block(nblk)

        # ---- gated = tile_G(avT) * sigT  (bf16) ----
        gat = []
        for g in range(KV):
            for i in range(G):
                t = sb.tile([P, ROWS], bf16, tag="gat", bufs=16,
                            name=f"gat{g}_{i}")
                nc.vector.tensor_mul(t[:], avT[g][:], sigT[4 * g + i][:])
                gat.append(t)

        # ---- out projection (bf16 weights staged from host) ----
        for nblk in range(4):
            pos = [ps.tile([P, CHUNK], f32, tag="ps", name=f"po{nblk}_{rt}")
                   for rt in range(SLOTS)]
            for cc in range(KT):
                bs = sb.tile([P, CHUNK], bf16, tag="woslab", bufs=8,
                             name=f"wob{nblk}_{cc}")
                nc.sync.dma_start(
                    bs[:], wo_d[cc * P:(cc + 1) * P,
                                nblk * CHUNK:(nblk + 1) * CHUNK])
                for rt in range(SLOTS):
                    nc.tensor.matmul(pos[rt][:],
                                     gat[cc][:, rt * P:(rt + 1) * P],
                                     bs[:], start=(cc == 0),
                                     stop=(cc == KT - 1))
            for rt in range(SLOTS):
                t = sb.tile([P, CHUNK], bf16, tag="oev", bufs=2,
                            name=f"oev{nblk}_{rt}")
                nc.scalar.copy(t[:], pos[rt][:])
                nc.sync.dma_start(
                    out_d[rt * P:(rt + 1) * P, nblk * CHUNK:(nblk + 1) * CHUNK],
                    t[:])

    nc.compile()
    return nc


def _split_hi_lo(x):
    import ml_dtypes
    hi = x.astype(ml_dtypes.bfloat16)
    lo = (x - hi.astype(np.float32)).astype(ml_dtypes.bfloat16)
    return np.ascontiguousarray(hi), np.ascontiguousarray(lo)


def kernel(hidden_states, cos, sin, attention_mask, Wq, Wk, Wv, Wo):
    import ml_dtypes
    from concourse.bass_utils import run_bass_kernel_spmd

    hidden_states = np.asarray(hidden_states, dtype=np.float32)
    cos = np.asarray(cos, dtype=np.float32)
    sin = np.asarray(sin, dtype=np.float32)
    mask = np.asarray(attention_mask, dtype=np.float32)[0, 0]
    Wq = np.asarray(Wq, dtype=np.float32)
    Wk = np.asarray(Wk, dtype=np.float32)
    Wv = np.asarray(Wv, dtype=np.float32)
    Wo = np.asarray(Wo, dtype=np.float32)

    classes = _mask_classes(mask)
    key = tuple(tuple(r) for r in classes)
    if key not in _CACHE:
        _CACHE[key] = _build(classes)
    nc = _CACHE[key]

    # weights: [v | k] column layout, host-side hi/lo bf16 split
    wkv = np.concatenate([Wv, Wk], axis=1)          # [HS, 2*KV*D]
    wkvH, wkvL = _split_hi_lo(wkv)
    # per-head interleave for the g-outer v pass: [g][whi 128 | wlo 128]
    wvP = np.concatenate(
        [np.concatenate([wkvH[:, g * P:(g + 1) * P],
                         wkvL[:, g * P:(g + 1) * P]], axis=1)
         for g in range(KV)], axis=1)
    wqg = np.ascontiguousarray(Wq[:, HS:]).astype(ml_dtypes.bfloat16)
    wo_b = Wo.astype(ml_dtypes.bfloat16)

    in_maps = []
    for core in range(NCORES):
        b, j = divmod(core, RANKS)
        blocks = [RANKS * k + j for k in range(SLOTS)]
        rows = np.concatenate([np.arange(bi * P, (bi + 1) * P) for bi in blocks])
        strips = []
        for k in range(SLOTS):
            for c in range(NCHUNK):
                if classes[k][c] == 2:
                    bi = RANKS * k + j
                    strips.append(mask[bi * P:(bi + 1) * P,
                                       c * CHUNK:(c + 1) * CHUNK])
        if not strips:
            strips.append(np.zeros((P, CHUNK), np.float32))
        hidT = np.ascontiguousarray(hidden_states[b][rows].T)
        hidH, hidL = _split_hi_lo(hidT)
        in_maps.append({
            "hidHL": np.ascontiguousarray(
                np.concatenate([hidH, hidL], axis=1)),
            "wvP": np.ascontiguousarray(wvP),
            "wkvH": wkvH,
            "wkvL": wkvL,
            "wqg": wqg,
            "wo": wo_b,
            "cosT": np.ascontiguousarray(cos[b][rows].T),
            "sinT": np.ascontiguousarray(sin[b][rows].T),
            "maskst": np.ascontiguousarray(np.stack(strips)),
        })

    res = run_bass_kernel_spmd(nc, in_maps, core_ids=list(range(NCORES)))

    out = np.empty((B, S, HS), np.float32)
    for core in range(NCORES):
        b, j = divmod(core, RANKS)
        o = res.results[core]["out"]
        for k in range(SLOTS):
            bi = RANKS * k + j
            out[b, bi * P:(bi + 1) * P, :] = o[k * P:(k + 1) * P, :].astype(
                np.float32)
    return out


# revision 21
# speedup vs baseline: 1.0073x; 1.0073x over previous
"""Trainium2 Bass kernel for nn_Attention_34351148434119 (8 NeuronCores).

Reference computation (faithful quirks included):
  q_proj = hid @ Wq; q, gate = split(q_proj)     # q is DEAD code downstream
  k = hid @ Wk; v = hid @ Wv                     # [B,KV,S,D]
  v = RoPE(v)  (k is NOT roped; q roped but unused)
  scores = (k @ v^T) * sqrt(D) + mask; attn = softmax_t(scores)   # per kv head
  out = (tile_G(attn @ v) * sigmoid(gate)) @ Wo

Sharding: core = b*4 + j  (b = batch, j = rank in 4-core batch group).
Per batch, S=2048 is split into 16 blocks of 128 rows; core j owns blocks
{j, 4+j, 8+j, 12+j} (slot k block = 4k+j) so every core has an identical
causal workload (uniform SPMD graph; per-core specialization only via
staged data).

Schedule (v4): hi/lo bf16 splits are precomputed on the HOST (hidden
staged as one [HS, 2*ROWS] hi|lo array; Wk/Wv as bf16 hi+lo pairs;
Wq-gate and Wo staged bf16) so projections are pure DMA->PE with ONE
coarse DMA per contraction tile (v3's fine-grained per-head slabs choked
the sync sequencer's ~0.6us/DMA issue rate).  v is projected first
(kk-outer), the four per-head fp32 d-major AllGathers fire ~50us in; the
k projection, gate block 0 and the per-head attention overlap the
collectives.  Gathered-v loads ride the gpsimd queue only.  Row-major v
for attn@v is rebuilt on-chip with bf16 PE transposes of a bf16 cast of
the gathered v.  Softmax keeps attn UNNORMALIZED through the transpose
and attn@v; the 1/sum is applied on the [d,s]-layout attn@v output via
PE-transpose of the reciprocal + gpsimd partition_broadcast.
Consecutive matmuls share the stationary operand where possible
([whi@hidH, whi@hidL, wlo@hidH] triple order skips one weight load).

Precision: logits have sigma~105 (SCALING MULTIPLIES by sqrt(D)), so
softmax is effectively near-argmax and bf16 anywhere in the k/v->scores
chain flips argmax rows (rel err ~0.1).  The k/v projections use the
hi/lo bf16 split (3 passes = ~16-bit effective mantissa at full bf16 PE
speed); scores run in native fp32; gate / attn@v / out-proj run bf16.
"""
import sys
import numpy as np

sys.path.insert(0, "/opt/trn_rl_repo")

B, S, HS = 2, 2048, 2048
H, KV, D = 16, 4, 128
G = H // KV
SCALING = float(D) ** 0.5
P = 128
NB = S // P            # 16 row blocks per batch
NCORES = 8
RANKS = 4              # cores per batch group
SLOTS = 4              # owned 128-row blocks per core
ROWS = SLOTS * P       # 512 rows per core
CHUNK = 512            # t-chunk = 4 t-tiles
NCHUNK = S // CHUNK    # 4
KT = HS // P           # 16 contraction tiles
NEG_THRESH = -1e8

_CACHE = {}


def _mask_classes(mask):
    """Classify each (s-slot k, t-chunk c) 512x512 region of the SxS mask.

    0 = skip (everything <= NEG_THRESH: contributes exact 0 after softmax)
    1 = plain (all zeros: no add needed)
    2 = add  (mixed: stage values and add on-chip)
    Slot k rows across all cores = blocks 4k..4k+3 = rows [512k, 512k+512).
    """
    cls = [[0] * NCHUNK for _ in range(SLOTS)]
    for k in range(SLOTS):
        for c in range(NCHUNK):
            reg = mask[512 * k:512 * (k + 1), 512 * c:512 * (c + 1)]
            if (reg <= NEG_THRESH).all():
                cls[k][c] = 0
            elif (reg == 0).all():
                cls[k][c] = 1
            else:
                cls[k][c] = 2
    ok = True
    for k in range(SLOTS):
        comp = [c for c in range(NCHUNK) if cls[k][c] != 0]
        # computed chunks must be a prefix starting at 0
        if comp != list(range(len(comp))) or 0 not in comp:
            ok = False
    if ok:
        # {k : chunk c computed} must be a suffix of slots for each c
        for c in range(NCHUNK):
            ks = [k for k in range(SLOTS) if cls[k][c] != 0]
            if ks != list(range(SLOTS - len(ks), SLOTS)):
                ok = False
    if not ok:
        # fully dense fallback: always correct for any mask
        cls = [[2] * NCHUNK for _ in range(SLOTS)]
    return cls


def _build(classes):
    from contextlib import ExitStack

    from concourse import bacc, mybir, tile
    from concourse.masks import make_identity

    f32 = mybir.dt.float32
    bf16 = mybir.dt.bfloat16
    Alu = mybir.AluOpType
    Act = mybir.ActivationFunctionType

    computed = [[c for c in range(NCHUNK) if classes[k][c] != 0] for k in range(SLOTS)]
    add_idx = {}
    for k in range(SLOTS):
        for c in range(NCHUNK):
            if classes[k][c] == 2:
                add_idx[(k, c)] = len(add_idx)
    n_add = max(len(add_idx), 1)

    nc = bacc.Bacc("TRN2", target_bir_lowering=False, debug=False,
                   num_devices=NCORES)

    # hidHL: per contraction tile kk, columns [hidH(512) | hidL(512)]
    hidHL_d = nc.declare_dram_parameter("hidHL", [HS, 2 * ROWS], bf16,
                                        isOutput=False)
    wvP_d = nc.declare_dram_parameter("wvP", [HS, 2 * KV * D], bf16,
                                      isOutput=False)
    wkvH_d = nc.declare_dram_parameter("wkvH", [HS, 2 * KV * D], bf16,
                                       isOutput=False)
    wkvL_d = nc.declare_dram_parameter("wkvL", [HS, 2 * KV * D], bf16,
                                       isOutput=False)
    wqg_d = nc.declare_dram_parameter("wqg", [HS, HS], bf16, isOutput=False)
    wo_d = nc.declare_dram_parameter("wo", [HS, HS], bf16, isOutput=False)
    cosT_d = nc.declare_dram_parameter("cosT", [D, ROWS], f32, isOutput=False)
    sinT_d = nc.declare_dram_parameter("sinT", [D, ROWS], f32, isOutput=False)
    mask_d = nc.declare_dram_parameter("maskst", [n_add, P, CHUNK], f32,
                                       isOutput=False)
    out_d = nc.declare_dram_parameter("out", [ROWS, HS], bf16, isOutput=True)

    rg = [[0, 1, 2, 3], [4, 5, 6, 7]]

    with tile.TileContext(nc) as tc, ExitStack() as ctx:
        sb = ctx.enter_context(tc.tile_pool(name="sb", bufs=2))
        ps = ctx.enter_context(tc.tile_pool(name="ps", bufs=8, space="PSUM"))
        dram = ctx.enter_context(tc.tile_pool(name="dram", bufs=1, space="DRAM"))

        # ---- constants ----
        id_f32 = sb.tile([P, P], f32, tag="c_idf")
        id_bf = sb.tile([P, P], bf16, tag="c_idb")
        make_identity(nc, id_f32[:])
        make_identity(nc, id_bf[:])
        cosT = sb.tile([D, ROWS], f32, tag="c_cos")
        sinT = sb.tile([D, ROWS], f32, tag="c_sin")
        msk = []
        for i in range(n_add):
            t = sb.tile([P, CHUNK], f32, tag="msk", bufs=n_add, name=f"msk{i}")
            nc.scalar.dma_start(t[:], mask_d[i, :, :])
            msk.append(t)

        # ---- resident hidden (hi|lo), first on the sync queue ----
        hid = []       # [128, 1024] per kk: cols [H(512) | L(512)], resident
        for kk in range(KT):
            hh = sb.tile([P, 2 * ROWS], bf16, tag="hid", bufs=KT,
                         name=f"hid{kk}")
            nc.sync.dma_start(hh[:], hidHL_d[kk * P:(kk + 1) * P, :])
            hid.append(hh)
            if kk == 3:
                nc.sync.dma_start(cosT[:], cosT_d[:, :])
                nc.sync.dma_start(sinT[:], sinT_d[:, :])

        # ---- v projection (g-OUTER, 3-pass hi/lo bf16) + per-head AG ----
        # wvP columns: per head g, 256 cols = [whi(128) | wlo(128)]
        vt_all = []
        for g in range(KV):
            pvg = ps.tile([P, ROWS], f32, tag="ps", name=f"pv{g}")
            for kk in range(KT):
                wv = sb.tile([P, 2 * P], bf16, tag="wvg", bufs=8,
                             name=f"wvg{g}_{kk}")
                eng = nc.sync if kk % 2 == 0 else nc.scalar
                eng.dma_start(wv[:], wvP_d[kk * P:(kk + 1) * P,
                                           g * 2 * P:(g + 1) * 2 * P])
                hH = hid[kk][:, 0:ROWS]
                hL = hid[kk][:, ROWS:2 * ROWS]
                # whi shared by the first two matmuls (saves a weight load)
                nc.tensor.matmul(pvg[:], wv[:, 0:P], hH,
                                 start=(kk == 0), stop=False)
                nc.tensor.matmul(pvg[:], wv[:, 0:P], hL,
                                 start=False, stop=False)
                nc.tensor.matmul(pvg[:], wv[:, P:2 * P], hH,
                                 start=False, stop=(kk == KT - 1))
            vr = sb.tile([P, ROWS], f32, tag="vraw", bufs=2, name=f"vr{g}")
            nc.scalar.copy(vr[:], pvg[:])
            rot = sb.tile([P, ROWS], f32, tag="vrot", bufs=2, name=f"rot{g}")
            nc.vector.tensor_scalar_mul(rot[0:64, :], vr[64:128, :], -1.0)
            nc.vector.tensor_copy(rot[64:128, :], vr[0:64, :])
            nc.vector.tensor_mul(vr[:], vr[:], cosT[:])
            nc.vector.tensor_mul(rot[:], rot[:], sinT[:])
            nc.vector.tensor_add(vr[:], vr[:], rot[:])
            vt_in = dram.tile([P, ROWS], f32, name=f"vtin{g}")
            nc.sync.dma_start(vt_in[:], vr[:])
            vt_out = dram.tile([RANKS * P, ROWS], f32, name=f"vtout{g}")
            nc.gpsimd.collective_compute(
                "AllGather", mybir.AluOpType.bypass, replica_groups=rg,
                ins=[vt_in.opt()], outs=[vt_out.opt()])
            vt_all.append(vt_out)

        # ---- k projection (kk-outer, 3-pass hi/lo bf16) ----
        pk = [ps.tile([P, ROWS], f32, tag="ps", name=f"pk{g}") for g in range(KV)]
        wqb0 = []
        for kk in range(KT):
            wk = sb.tile([P, 2 * KV * D], bf16, tag="wks", bufs=4,
                         name=f"wks{kk}")
            nc.sync.dma_start(wk[:, 0:KV * D],
                              wkvH_d[kk * P:(kk + 1) * P, KV * D:])
            nc.sync.dma_start(wk[:, KV * D:],
                              wkvL_d[kk * P:(kk + 1) * P, KV * D:])
            if kk < 8:
                bs = sb.tile([P, CHUNK], bf16, tag="wqslab", bufs=8,
                             name=f"wqb0_{kk}")
                nc.sync.dma_start(bs[:], wqg_d[kk * P:(kk + 1) * P, 0:CHUNK])
                wqb0.append(bs)
            hH = hid[kk][:, 0:ROWS]
            hL = hid[kk][:, ROWS:2 * ROWS]
            for g in range(KV):
                hi = slice(g * P, (g + 1) * P)
                lo = slice(KV * D + g * P, KV * D + (g + 1) * P)
                nc.tensor.matmul(pk[g][:], wk[:, hi], hH,
                                 start=(kk == 0), stop=False)
                nc.tensor.matmul(pk[g][:], wk[:, hi], hL,
                                 start=False, stop=False)
                nc.tensor.matmul(pk[g][:], wk[:, lo], hH,
                                 start=False, stop=(kk == KT - 1))
        kT = []
        for g in range(KV):
            t = sb.tile([P, ROWS], f32, tag="kT", bufs=KV, name=f"kT{g}")
            nc.scalar.mul(t[:], pk[g][:], SCALING)
            kT.append(t)

        # ---- gate matmul helper (bf16 weights staged from host) ----
        sigT = [None] * 16

        def gate_block(nblk):
            pgs = [ps.tile([P, ROWS], f32, tag="ps", name=f"pg{nblk}_{m}")
                   for m in range(4)]
            for kk in range(KT):
                if nblk == 0 and kk < 8:
                    bs = wqb0[kk]
                else:
                    bs = sb.tile([P, CHUNK], bf16, tag="wqslab", bufs=8,
                                 name=f"wqbx{nblk}_{kk}")
                    nc.sync.dma_start(
                        bs[:], wqg_d[kk * P:(kk + 1) * P,
                                     nblk * CHUNK:(nblk + 1) * CHUNK])
                for m in range(4):
                    nc.tensor.matmul(pgs[m][:], bs[:, m * P:(m + 1) * P],
                                     hid[kk][:, 0:ROWS], start=(kk == 0),
                                     stop=(kk == KT - 1))
            for m in range(4):
                t = sb.tile([P, ROWS], bf16, tag="sigT", bufs=16,
                            name=f"sig{nblk}_{m}")
                nc.scalar.activation(t[:], pgs[m][:], Act.Sigmoid)
                sigT[nblk * 4 + m] = t

        # gate block 0 fills the PE while the first AllGather completes
        gate_block(0)

        # ---- attention per kv head, software-pipelined ----
        # Stages per unit u=(g,k):
        #   S1: score matmuls (PE) + mask add + chunk maxes + min chain (DVE)
        #   S2: exp wave + sum chain (ACT only)
        #   S3: PE transposes + PSUM->SBUF copies into attnT
        #   S4 (per g): attn@v, 1/sum reciprocal+broadcast, normalized avT
        # Emission is pipelined (S1(u) | S2(u-1) | S3(u-2)) so each engine's
        # FIFO sees early-stage ops of later units BEFORE late-stage ops of
        # earlier units -- without this the 16 units run back-to-back
        # serially (~11us latency each).
        avT = [None] * KV
        order = sorted(range(SLOTS), key=lambda k: -len(computed[k]))

        def load_v(g):
            # load gathered v (d-major fp32): vtc[c] = [128 d, 512 t]
            # t-block 4c+r lives in rank r's AG block at column-slot c.
            vtc = []
            vtcb = []
            for c in range(NCHUNK):
                t = sb.tile([P, CHUNK], f32, tag="vtc", bufs=8,
                            name=f"vtc{g}_{c}")
                for r in range(RANKS):
                    nc.gpsimd.dma_start(
                        t[:, r * P:(r + 1) * P],
                        vt_all[g][r * P:(r + 1) * P, c * P:(c + 1) * P])
                vtc.append(t)
                tb = sb.tile([P, CHUNK], bf16, tag="vtcb", bufs=8,
                             name=f"vtcb{g}_{c}")
                if (g + c) % 2:
                    nc.scalar.copy(tb[:], t[:])
                else:
                    nc.vector.tensor_copy(tb[:], t[:])
                vtcb.append(tb)
            # row-major bf16 v via PE transpose of the bf16 cast
            vrg = []
            for c in range(NCHUNK):
                tp = ps.tile([P, CHUNK], bf16, tag="ps", name=f"vtp{g}_{c}")
                for r in range(RANKS):
                    nc.tensor.transpose(tp[:, r * P:(r + 1) * P],
                                        vtcb[c][:, r * P:(r + 1) * P], id_bf[:])
                for r in range(RANKS):
                    t = sb.tile([P, P], bf16, tag="vrg", bufs=4 * NCHUNK * 2,
                                name=f"vrg{g}_{4 * c + r}")
                    if r % 2:
                        nc.scalar.copy(t[:], tp[:, r * P:(r + 1) * P])
                    else:
                        nc.vector.tensor_copy(t[:], tp[:, r * P:(r + 1) * P])
                    vrg.append(t)
            return vtc, vtcb, vrg

        def s1_scores(g, k, vtc):
            comp = computed[k]
            pscs = []
            cms = []
            for ci, c in enumerate(comp):
                psc = ps.tile([P, CHUNK], f32, tag="ps",
                              name=f"psc{g}_{k}_{ci}")
                nc.tensor.matmul(psc[:], kT[g][:, k * P:(k + 1) * P],
                                 vtc[c][:], start=True, stop=True)
                if classes[k][c] == 2:
                    nc.vector.tensor_add(psc[:], psc[:],
                                         msk[add_idx[(k, c)]][:])
                cm = sb.tile([P, 1], f32, tag="stat", bufs=96,
                             name=f"cm{g}_{k}_{ci}")
                nc.vector.tensor_reduce(cm[:], psc[:], mybir.AxisListType.X,
                                        Alu.max, negate=True)
                pscs.append(psc)
                cms.append(cm)
            mneg = cms[0]   # -max
            for ci in range(1, len(comp)):
                mnew = sb.tile([P, 1], f32, tag="stat", bufs=96,
                               name=f"mn{g}_{k}_{ci}")
                nc.vector.tensor_tensor(mnew[:], mneg[:], cms[ci][:], Alu.min)
                mneg = mnew
            return pscs, mneg

        def s2_exp(g, k, pscs, mneg):
            comp = computed[k]
            nchk = len(comp)
            attn = sb.tile([P, CHUNK * nchk], bf16, tag="attn", bufs=3,
                           padded_shape=[P, CHUNK * NCHUNK],
                           name=f"attn{g}_{k}")
            tot = None
            for ci in range(nchk):
                csum = sb.tile([P, 1], f32, tag="stat", bufs=96,
                               name=f"cs{g}_{k}_{ci}")
                nc.scalar.activation(attn[:, ci * CHUNK:(ci + 1) * CHUNK],
                                     pscs[ci][:], Act.Exp, bias=mneg[:],
                                     accum_out=csum[:])
                if tot is None:
                    tot = csum
                else:
                    t2 = sb.tile([P, 1], f32, tag="stat", bufs=96,
                                 name=f"tt{g}_{k}_{ci}")
                    nc.scalar.add(t2[:], csum[:], tot[:])
                    tot = t2
            return attn, tot

        def s3_transpose(g, k, attn, attnT):
            comp = computed[k]
            for ci, c in enumerate(comp):
                tp = ps.tile([P, CHUNK], bf16, tag="ps",
                             name=f"atp{g}_{k}_{ci}")
                for i in range(4):
                    nc.tensor.transpose(
                        tp[:, i * P:(i + 1) * P],
                        attn[:, ci * CHUNK + i * P:ci * CHUNK + (i + 1) * P],
                        id_bf[:])
                for i in range(4):
                    bi = 4 * c + i
                    if (ci + i) % 2:
                        nc.scalar.copy(attnT[bi][:, k * P:(k + 1) * P],
                                       tp[:, i * P:(i + 1) * P])
                    else:
                        nc.vector.tensor_copy(attnT[bi][:, k * P:(k + 1) * P],
                                              tp[:, i * P:(i + 1) * P])

        def s4_av(g, attnT, vrg, tots):
            # attn @ v  ->  pav [128 d, 512 s]  (unnormalized)
            pav = ps.tile([P, ROWS], f32, tag="ps", name=f"pav{g}")
            first = True
            for bi in range(NB):
                ks = [k for k in range(SLOTS) if (bi // RANKS) in computed[k]]
                if not ks:
                    continue
                kmin = ks[0]
                nc.tensor.matmul(pav[:, kmin * P:ROWS], vrg[bi][:],
                                 attnT[bi][:, kmin * P:ROWS],
                                 start=first, stop=(bi == NB - 1))
                first = False
            # 1/sum: PE-transpose rinv [128,1] -> [1,128], gpsimd broadcast
            # to [128,128], multiply into the PSUM->SBUF copy of attn@v.
            t = sb.tile([P, ROWS], bf16, tag="avT", bufs=KV, name=f"avT{g}")
            for k in range(SLOTS):
                rinv = sb.tile([P, 1], f32, tag="stat", bufs=96,
                               name=f"rinv{g}_{k}")
                nc.vector.reciprocal(rinv[:], tots[k][:])
                rtp = ps.tile([1, P], f32, tag="ps", name=f"rtp{g}_{k}")
                nc.tensor.transpose(rtp[:], rinv[:], id_f32[:])
                rrow = sb.tile([1, P], f32, tag="rrow", bufs=8,
                               name=f"rrow{g}_{k}")
                nc.vector.tensor_copy(rrow[:], rtp[:])
                rbs = sb.tile([P, P], f32, tag="rbs", bufs=4, name=f"rbs{g}_{k}")
                nc.gpsimd.partition_broadcast(rbs[:], rrow[:])
                nc.vector.tensor_mul(t[:, k * P:(k + 1) * P],
                                     pav[:, k * P:(k + 1) * P], rbs[:])
            avT[g] = t

        units = [(g, k) for g in range(KV) for k in order]
        NU = len(units)
        vload = [None] * KV
        vload[0] = load_v(0)
        attnTs = {}
        s1st = {}
        s2st = {}
        tots = {}
        for i in range(NU + 2):
            if i < NU:
                g, k = units[i]
                if k == order[0]:
                    attnTs[g] = [
                        sb.tile([P, ROWS], bf16, tag="attnT", bufs=NB,
                                name=f"attnT{g}_{bi}") for bi in range(NB)]
                if k == order[-1] and g + 1 < KV:
                    vload[g + 1] = load_v(g + 1)
                s1st[i] = s1_scores(g, k, vload[g][0])
            if 1 <= i <= NU:
                g, k = units[i - 1]
                s2st[i - 1] = s2_exp(g, k, *s1st[i - 1])
            if 2 <= i <= NU + 1:
                g, k = units[i - 2]
                attn, tot = s2st[i - 2]
                s3_transpose(g, k, attn, attnTs[g])
                tots.setdefault(g, {})[k] = tot
                if k == order[-1]:
                    s4_av(g, attnTs[g], vload[g][2], tots[g])

        # ---- remaining gate blocks ----
        for nblk in range(1, 4):
            gate_block(nblk)

        # ---- gated = tile_G(avT) * sigT  (bf16) ----
        gat = []
        for g in range(KV):
            for i in range(G):
                t = sb.tile([P, ROWS], bf16, tag="gat", bufs=16,
                            name=f"gat{g}_{i}")
                nc.vector.tensor_mul(t[:], avT[g][:], sigT[4 * g + i][:])
                gat.append(t)

        # ---- out projection (bf16 weights staged from host) ----
        for nblk in range(4):
            pos = [ps.tile([P, CHUNK], f32, tag="ps", name=f"po{nblk}_{rt}")
                   for rt in range(SLOTS)]
            for cc in range(KT):
                bs = sb.tile([P, CHUNK], bf16, tag="woslab", bufs=8,
                             name=f"wob{nblk}_{cc}")
                nc.sync.dma_start(
                    bs[:], wo_d[cc * P:(cc + 1) * P,
                                nblk * CHUNK:(nblk + 1) * CHUNK])
                for rt in range(SLOTS):
                    nc.tensor.matmul(pos[rt][:],
                                     gat[cc][:, rt * P:(rt + 1) * P],
                                     bs[:], start=(cc == 0),
                                     stop=(cc == KT - 1))
            for rt in range(SLOTS):
                t = sb.tile([P, CHUNK], bf16, tag="oev", bufs=2,
                            name=f"oev{nblk}_{rt}")
                nc.scalar.copy(t[:], pos[rt][:])
                nc.sync.dma_start(
                    out_d[rt * P:(rt + 1) * P, nblk * CHUNK:(nblk + 1) * CHUNK],
                    t[:])

    nc.compile()
    return nc


def _split_hi_lo(x):
    import ml_dtypes
    hi = x.astype(ml_dtypes.bfloat16)
    lo = (x - hi.astype(np.float32)).astype(ml_dtypes.bfloat16)
    return np.ascontiguousarray(hi), np.ascontiguousarray(lo)


def kernel(hidden_states, cos, sin, attention_mask, Wq, Wk, Wv, Wo):
    import ml_dtypes
    from concourse.bass_utils import run_bass_kernel_spmd

    hidden_states = np.asarray(hidden_states, dtype=np.float32)
    cos = np.asarray(cos, dtype=np.float32)
    sin = np.asarray(sin, dtype=np.float32)
    mask = np.asarray(attention_mask, dtype=np.float32)[0, 0]
    Wq = np.asarray(Wq, dtype=np.float32)
    Wk = np.asarray(Wk, dtype=np.float32)
    Wv = np.asarray(Wv, dtype=np.float32)
    Wo = np.asarray(Wo, dtype=np.float32)

    classes = _mask_classes(mask)
    key = tuple(tuple(r) for r in classes)
    if key not in _CACHE:
        _CACHE[key] = _build(classes)
    nc = _CACHE[key]

    # weights: [v | k] column layout, host-side hi/lo bf16 split
    wkv = np.concatenate([Wv, Wk], axis=1)          # [HS, 2*KV*D]
    wkvH, wkvL = _split_hi_lo(wkv)
    # per-head interleave for the g-outer v pass: [g][whi 128 | wlo 128]
    wvP = np.concatenate(
        [np.concatenate([wkvH[:, g * P:(g + 1) * P],
                         wkvL[:, g * P:(g + 1) * P]], axis=1)
         for g in range(KV)], axis=1)
    wqg = np.ascontiguousarray(Wq[:, HS:]).astype(ml_dtypes.bfloat16)
    wo_b = Wo.astype(ml_dtypes.bfloat16)

    in_maps = []
    for core in range(NCORES):
        b, j = divmod(core, RANKS)
        blocks = [RANKS * k + j for k in range(SLOTS)]
        rows = np.concatenate([np.arange(bi * P, (bi + 1) * P) for bi in blocks])
        strips = []
        for k in range(SLOTS):
            for c in range(NCHUNK):
                if classes[k][c] == 2:
                    bi = RANKS * k + j
                    strips.append(mask[bi * P:(bi + 1) * P,
                                       c * CHUNK:(c + 1) * CHUNK])
        if not strips:
            strips.append(np.zeros((P, CHUNK), np.float32))
        hidT = np.ascontiguousarray(hidden_states[b][rows].T)
        hidH, hidL = _split_hi_lo(hidT)
        in_maps.append({
            "hidHL": np.ascontiguousarray(
                np.concatenate([hidH, hidL], axis=1)),
            "wvP": np.ascontiguousarray(wvP),
            "wkvH": wkvH,
            "wkvL": wkvL,
            "wqg": wqg,
            "wo": wo_b,
            "cosT": np.ascontiguousarray(cos[b][rows].T),
            "sinT": np.ascontiguousarray(sin[b][rows].T),
            "maskst": np.ascontiguousarray(np.stack(strips)),
        })

    res = run_bass_kernel_spmd(nc, in_maps, core_ids=list(range(NCORES)))

    out = np.empty((B, S, HS), np.float32)
    for core in range(NCORES):
        b, j = divmod(core, RANKS)
        o = res.results[core]["out"]
        for k in range(SLOTS):
            bi = RANKS * k + j
            out[b, bi * P:(bi + 1) * P, :] = o[k * P:(k + 1) * P, :].astype(
                np.float32)
    return out
